# revision 24
# baseline (speedup 1.0000x reference)
"""BitLinear (ternary-weight linear) on 8 Trainium2 NeuronCores.

Computation: out = x @ (clip(round(w/s), -1, 1) * s).T + bias, where s is
the per-output-row lower median of |w|.

Strategy
- Host side: compute the per-row scale s (exact reference semantics via
  np.partition) and the ternary weights wq in {-1, 0, 1}; quantize x into
  two E4M3 fp8 planes x_hi = fp8(x), x_lo = fp8(x - x_hi). The device
  accumulates (x_hi + x_lo) @ wq exactly (fp8 E4M3 values are exact in
  the PE's e6m3 multiplier; ternary weights are exact in E4M3), so the
  only error is the double-fp8 representation of x (~1e-3 relative).
- Sharding: data-parallel over tokens. Each core owns 1024 of the 8192
  tokens; both x planes (8.4 MB fp8) sit resident in SBUF while the
  ternary weight (16.8 MB fp8) streams through once. No collectives.
- Matmul: fp8 DoubleRow perf mode - each instruction contracts K=256
  (128 partitions x 2 planes) over a 512-wide moving dim, measured at
  ~205 ns vs ~224 ns for the same-shape f32r matmul with half the K.
  2048 DoubleRow matmuls per core vs 2048 f32r matmuls for the f32r
  baseline => ~2x arithmetic throughput.
- Contraction layout: chunk c of 16 covers original k rows
  [(2c)*128, (2c+2)*128); plane i pairs rows (2c+i)*128+p. The hi and
  lo x planes are separate passes over the same weight chunk, so each
  streamed weight block is reused by up to 16 matmuls (2 passes x 8
  token blocks). M_LO < 16 drops the lo pass for chunks c >= M_LO
  (partial correction; error ~2.65e-2 * sqrt((16-M_LO)/16); M_LO=12
  gives 1.329e-2 measured, well under the 2e-2 gate, and cuts matmuls
  from 2048 to 1792 per core).
- DMA: one dma_start stripes across all 16 SDMA engines, so transfers
  are kept >= 1 MB: x lands in 8 x 1 MB chunks (double-buffered across
  reps), each feature block's weights in one contiguous 2 MB transfer
  (p-major DRAM layout) on the dedicated gpsimd/scalar queues
  (alternating); x loads and output stores share the sync queue.
  Output is stored as bf16 (halves store traffic; adds ~0.2% error in
  quadrature -> 1.349e-2 total) and upcast to f32 on the host.
- Per core: psum tiles [128 tokens x 512 features] x 8 banks accumulate
  over the full contraction; DVE epilogue fuses scale+bias while copying
  PSUM -> SBUF.
"""

import os
import sys

import numpy as np

for _p in ("/opt/trn_rl_repo", "/opt/pypackages"):
    if os.path.isdir(_p) and _p not in sys.path:
        sys.path.append(_p)

N_CORES = 8
B, S, IN_F, OUT_F = 4, 2048, 4096, 4096
TOK = B * S                # 8192 tokens total
TPC = TOK // N_CORES       # 1024 tokens per core
CH = IN_F // 256           # 16 contraction chunks of 256 (2 planes x 128)
FBW = 512                  # psum tile free width (one PSUM bank of fp32)
FB = OUT_F // FBW          # 8 feature blocks
TB = TPC // 128            # 8 token blocks per core
M_LO = 11                  # chunks receiving the x_lo correction pass (<= CH)
X_CHUNKS = 8               # DMAs used to land the resident x planes (1 MB each)

_CACHE = {}


def _patched_tile_context(nc):
    """TileContext subclass for this container's walrus, which rejects
    instructions carrying more than one sync-wait command. Tile's wait
    assignment (and its tail drain) can attach several; after scheduling,
    move the extras onto same-engine no-ops inserted just before the
    instruction (same program point, identical semantics)."""
    import concourse.mybir as mybir
    import concourse.tile as tile

    def _split_multi_waits(nc):
        for f in nc.m.functions:
            for blk in f.blocks:
                out = []
                changed = False
                for inst in blk.instructions:
                    si = inst.sync_info
                    waits = list(si.on_wait) if si and si.on_wait else []
                    cap = 2 if isinstance(inst, mybir.InstEventSemaphore) else 1
                    if len(waits) > cap:
                        changed = True
                        for w in waits[:-cap]:
                            nop = mybir.InstNoOp(
                                name=f"I-waitsplit-{nc.next_id()}", ins=[], outs=[]
                            )
                            nop.engine = inst.engine
                            nop.sync_info = mybir.SyncInfo(on_wait=[w], on_update=[])
                            out.append(nop)
                        inst.sync_info = mybir.SyncInfo(
                            on_wait=waits[-cap:], on_update=list(si.on_update or [])
                        )
                    out.append(inst)
                if changed:
                    blk.instructions = out

    class PatchedTileContext(tile.TileContext):
        def schedule_and_allocate(self):
            result = super().schedule_and_allocate()
            _split_multi_waits(self.nc)
            return result

    return PatchedTileContext(nc)


def _build_nc(reps=1):
    import concourse.bass as bass
    import concourse.mybir as mybir

    F32 = mybir.dt.float32
    F8 = mybir.dt.float8e4
    DR = mybir.MatmulPerfMode.DoubleRow

    nc = bass.Bass()
    # x planes: [partition, pass(hi/lo), chunk, plane, token]
    xt8 = nc.declare_dram_parameter(
        "xt8", [128, 2 * CH * 2 * TPC], F8, isOutput=False
    )
    wq8 = nc.declare_dram_parameter(
        "wq8", [FB, 128, CH, 2, FBW], F8, isOutput=False
    )
    BF16 = mybir.dt.bfloat16
    s_bc = nc.declare_dram_parameter("s_bc", [128, OUT_F], F32, isOutput=False)
    b_bc = nc.declare_dram_parameter("b_bc", [128, OUT_F], F32, isOutput=False)
    out = nc.declare_dram_parameter("out", [TPC, OUT_F], BF16, isOutput=True)

    with _patched_tile_context(nc) as tc:
        with tc.tile_pool(name="xp", bufs=2) as xp, \
             tc.tile_pool(name="cp", bufs=1) as cp, \
             tc.tile_pool(name="wp", bufs=2) as wp, \
             tc.tile_pool(name="op", bufs=6) as op, \
             tc.tile_pool(name="pp", bufs=1, space="PSUM") as pp:

            s_sb = cp.tile([128, OUT_F], F32, name="s_sb")
            nc.sync.dma_start(s_sb[:], s_bc[:])
            b_sb = cp.tile([128, OUT_F], F32, name="b_sb")
            nc.sync.dma_start(b_sb[:], b_bc[:])

            for rep in range(reps):
                x_sb = xp.tile(
                    [128, 2, CH, 2, TPC], F8, name="x_sb", tag="x_sb"
                )
                x_flat = x_sb[:].opt({0})
                cw = (2 * CH * 2 * TPC) // X_CHUNKS
                for ci in range(X_CHUNKS):
                    nc.sync.dma_start(
                        x_flat[:, ci * cw:(ci + 1) * cw],
                        xt8[:, ci * cw:(ci + 1) * cw],
                    )
                for fb in range(FB):
                    ptiles = [
                        pp.tile([128, FBW], F32, name=f"ps{tb}", tag=f"ps{tb}")
                        for tb in range(TB)
                    ]
                    wt = wp.tile([128, CH, 2, FBW], F8, name="wt", tag="wt")
                    dma_q = nc.gpsimd if fb % 2 == 0 else nc.scalar
                    dma_q.dma_start(wt[:], wq8[fb])
                    last_c = CH - 1
                    last_pass = (2 if last_c < M_LO else 1) - 1
                    # tb outer: each token block's accumulation group runs as
                    # consecutive matmuls into ONE psum bank (no per-MM bank
                    # switching), and its epilogue overlaps the next groups.
                    for tb in range(TB):
                        for c in range(CH):
                            n_pass = 2 if c < M_LO else 1
                            for pss in range(n_pass):
                                nc.tensor.matmul(
                                    ptiles[tb][:],
                                    lhsT=x_sb[:, pss, c, :, tb * 128:(tb + 1) * 128],
                                    rhs=wt[:, c],
                                    start=(c == 0 and pss == 0),
                                    stop=(c == last_c and pss == last_pass),
                                    perf_mode=DR,
                                )
                        ot = op.tile([128, FBW], BF16, name="ot", tag="ot")
                        nc.vector.tensor_tensor(
                            ot[:], ptiles[tb][:],
                            s_sb[:, fb * FBW:(fb + 1) * FBW], mybir.AluOpType.mult,
                        )
                        nc.vector.tensor_tensor(
                            ot[:], ot[:],
                            b_sb[:, fb * FBW:(fb + 1) * FBW], mybir.AluOpType.add,
                        )
                        nc.sync.dma_start(
                            out[tb * 128:(tb + 1) * 128, fb * FBW:(fb + 1) * FBW],
                            ot[:],
                        )
    return nc


def _get_nc():
    if "nc" not in _CACHE:
        _CACHE["nc"] = _build_nc()
    return _CACHE["nc"]


def _get_runner():
    """Jitted SPMD executor for the prebuilt Bass module, traced once and
    cached. Inputs are global arrays sharded on axis 0 over the 8 cores;
    output zero-buffers are generated on-device and donated."""
    if "runner" in _CACHE:
        return _CACHE["runner"]
    import jax
    import jax.numpy as jnp
    from jax.experimental.shard_map import shard_map
    from jax.sharding import Mesh, NamedSharding, PartitionSpec

    import concourse.mybir as mybir
    from concourse import bass2jax

    nc = _get_nc()
    assert nc.dbg_addr is None
    bass2jax.install_neuronx_cc_hook()

    partition_name = (
        nc.partition_id_tensor.name if nc.partition_id_tensor else None
    )
    in_names, out_names, out_avals = [], [], []
    for alloc in nc.m.functions[0].allocations:
        if not isinstance(alloc, mybir.MemoryLocationSet):
            continue
        name = alloc.memorylocations[0].name
        if alloc.kind == "ExternalInput":
            if name != partition_name:
                in_names.append(name)
        elif alloc.kind == "ExternalOutput":
            out_names.append(name)
            out_avals.append(
                jax.core.ShapedArray(
                    tuple(alloc.tensor_shape), mybir.dt.np(alloc.dtype)
                )
            )
    n_params, n_outs = len(in_names), len(out_names)
    all_in_names = tuple(
        in_names + out_names + ([partition_name] if partition_name else [])
    )

    def _body(*args):
        operands = list(args)
        if partition_name is not None:
            operands.append(bass2jax.partition_id_tensor())
        outs = bass2jax._bass_exec_p.bind(
            *operands,
            out_avals=tuple(out_avals),
            in_names=all_in_names,
            out_names=tuple(out_names),
            lowering_input_output_aliases=(),
            sim_require_finite=True,
            sim_require_nnan=True,
            nc=nc,
        )
        return tuple(outs)

    devices = jax.devices()[:N_CORES]
    mesh = Mesh(np.asarray(devices), ("core",))
    sharding = NamedSharding(mesh, PartitionSpec("core"))
    in_specs = (PartitionSpec("core"),) * (n_params + n_outs)
    out_specs = (PartitionSpec("core"),) * n_outs
    donate = tuple(range(n_params, n_params + n_outs))
    sharded = jax.jit(
        shard_map(
            _body, mesh=mesh, in_specs=in_specs, out_specs=out_specs,
            check_rep=False,
        ),
        donate_argnums=donate,
        keep_unused=True,
    )
    zeros_fn = jax.jit(
        lambda: tuple(
            jnp.zeros((N_CORES * a.shape[0], *a.shape[1:]), a.dtype)
            for a in out_avals
        ),
        out_shardings=(sharding,) * n_outs,
    )
    runner = dict(
        in_names=in_names, out_names=out_names, sharded=sharded,
        zeros_fn=zeros_fn, sharding=sharding,
    )
    _CACHE["runner"] = runner
    return runner


def _prep_inputs(x, weight, bias):
    """Host-side quantization, layout, and per-core sharding. Returns the
    global (axis-0 core-sharded) input arrays in runner order."""
    import ml_dtypes

    E4M3 = ml_dtypes.float8_e4m3

    x = np.asarray(x, dtype=np.float32)
    weight = np.asarray(weight, dtype=np.float32)
    bias = np.asarray(bias, dtype=np.float32)

    # Ternary quantization (matches the reference bit-for-bit): per-row
    # lower median of |w|, floored at 1e-12; wq = clip(round(w/s), -1, 1).
    mid = (IN_F - 1) // 2
    s = np.partition(np.abs(weight), mid, axis=1)[:, mid]
    s = np.maximum(s, np.float32(1e-12)).astype(np.float32)
    wq = np.clip(np.round(weight / s[:, None]), -1.0, 1.0).astype(np.float32)

    # weight layout [FB, 128, CH, 2, FBW]: row k=(2c+i)*128+p, col fb*FBW+o
    w5 = np.ascontiguousarray(
        wq.T.reshape(CH, 2, 128, FB, FBW).transpose(3, 2, 0, 1, 4)
    ).astype(E4M3)

    # x planes: hi = fp8(x), lo = fp8(x - hi); layout per core
    # [128 p, pass, chunk, plane, token], flattened to [128, 65536]
    x2 = x.reshape(N_CORES, TPC, IN_F)
    x_hi = x2.astype(E4M3)
    x_lo = (x2 - x_hi.astype(np.float32)).astype(E4M3)
    planes = np.stack([x_hi, x_lo], axis=1)  # [core, pass, tok, IN_F]
    # k = (2c+i)*128+p -> [core, pass, tok, CH, 2, 128]
    planes = planes.reshape(N_CORES, 2, TPC, CH, 2, 128)
    xt_all = np.ascontiguousarray(
        planes.transpose(0, 5, 1, 3, 4, 2)  # core, p, pass, c, i, tok
    ).reshape(N_CORES * 128, 2 * CH * 2 * TPC)

    s_h = np.ascontiguousarray(np.broadcast_to(s, (128, OUT_F)))
    b_h = np.ascontiguousarray(np.broadcast_to(bias, (128, OUT_F)))

    per_name = {
        "xt8": xt_all,
        "wq8": np.ascontiguousarray(
            np.broadcast_to(w5, (N_CORES, *w5.shape))
        ).reshape(N_CORES * FB, 128, CH, 2, FBW),
        "s_bc": np.ascontiguousarray(
            np.broadcast_to(s_h, (N_CORES, 128, OUT_F))
        ).reshape(N_CORES * 128, OUT_F),
        "b_bc": np.ascontiguousarray(
            np.broadcast_to(b_h, (N_CORES, 128, OUT_F))
        ).reshape(N_CORES * 128, OUT_F),
    }
    runner = _get_runner()
    return [per_name[n] for n in runner["in_names"]]


def _execute(dev_or_np_inputs):
    runner = _get_runner()
    zeros = runner["zeros_fn"]()
    outs = runner["sharded"](*dev_or_np_inputs, *zeros)
    return outs


def kernel(x, weight, bias):
    global_inputs = _prep_inputs(x, weight, bias)
    outs = _execute(global_inputs)
    out_name_idx = _get_runner()["out_names"].index("out")
    out = np.asarray(outs[out_name_idx])  # [TOK, OUT_F] bf16, token-sharded
    return out.astype(np.float32).reshape(B, S, OUT_F)
